# revision 73
# baseline (speedup 1.0000x reference)
"""CrossAttentionBlock kernel for 8 Trainium2 NeuronCores.

Reference computation (per batch b):
    q = x @ Wq;  k,v = y @ Wkv;  per head: softmax(q k^T / sqrt(dk)) v;
    out = concat_heads @ Wproj + bproj

Sharding: 8 cores = 2 batches x 4 head-groups (4 heads each). Each core
computes the partial output contribution of its 4 heads for its batch;
the host sums the 4 partials per batch and adds the bias.

Pipelined single-pass schedule per core (activations/weights in bf16):
    prologue:  y is DMA'd in 4 column chunks; K/V projections trail the
               chunks so PE starts ~5us in. x/wq follow; Q proj (it=0).
    phase C:   per (pair, it): per jt chunk of 128 keys:
                 scores matmul (PSUM, 2 heads) -> exp on Act engine (bf16 P)
                 -> PV matmuls with P as the stationary operand, so the
                 output lands as [128 q, 65] per (head, q-chunk); column 64
                 (ones-augmented V) is the softmax denominator.
               The phase C loop is software-pipelined ACROSS phases: the
               last 3 PV steps + normalization of phase p are emitted inside
               phase p+1's loop, and tail work (PE transposes of O back to
               [d, q], Q proj for it+1, out-proj tiles for it-1) drains
               through a thunk queue, one slot per jt step.
    normalize: reciprocal of column 64 is a per-partition scalar ->
               tensor_scalar multiply (no cross-partition broadcast).
    phase D:   outT[ct, it] = wp.T @ otn, staged per i-tile into one SBUF
               buffer and stored with a single DMA (bf16 partials; the host
               accumulates the 4 head-group partials in fp32).
"""

import numpy as np
import ml_dtypes

import concourse.bass as bass
import concourse.tile as tile
from concourse import bacc, mybir
from concourse.bass_utils import run_bass_kernel_spmd

B, LQ, LKV = 2, 2048, 2048
C, CTX, H, DK = 1024, 768, 16, 64
SCALE = DK ** (-0.5)

F32 = mybir.dt.float32
BF16 = mybir.dt.bfloat16
BF16NP = ml_dtypes.bfloat16

PVLAG = 7   # pv(jt) runs PVLAG jt-steps behind scores(jt). The pt ring
# must hold >= 2*PVLAG tiles: the next phase's prologue exps reuse ring
# slots, and any slot whose consumer (a deferred tail pv) has not been
# EMITTED yet would be silently clobbered -- the tile framework can only
# order consumers that already exist in program order.

# Schraudolph fast-exp on DVE for a few jt chunks per phase: bf16 bits of
# 2^t are built as int16(t*128 + 16256 - C) in one tensor_scalar (the HW
# float->int16 convert rounds to nearest). Offloading 4/16 of the exp work
# from the saturated Act engine costs ~2% extra error on the offloaded
# keys' softmax weights, ~9e-3 end to end (gate is 2e-2).
FEXP_JT = (4, 8, 11, 14)
FEXP_C1 = float(SCALE * np.log2(np.e) * 128.0)
FEXP_B0 = 16256.0 - 5.5
I16 = mybir.dt.int16


def build_kernel(lq=LQ, lkv=LKV, c=C, ctx=CTX, hd=256, debug_taps=False):
    """One core's program: 4 heads (2 pairs) of cross-attention + partial proj."""
    nc = bacc.Bacc("TRN2", target_bir_lowering=False, debug=False)

    xT = nc.dram_tensor("xT", [c, lq], BF16, kind="ExternalInput").ap()
    yT = nc.dram_tensor("yT", [ctx, lkv], BF16, kind="ExternalInput").ap()
    wq = nc.dram_tensor("wq", [c, hd], BF16, kind="ExternalInput").ap()
    wk = nc.dram_tensor("wk", [ctx, hd], BF16, kind="ExternalInput").ap()
    wv = nc.dram_tensor("wv", [ctx, hd], BF16, kind="ExternalInput").ap()
    wp = nc.dram_tensor("wp", [hd, c], BF16, kind="ExternalInput").ap()
    ident = nc.dram_tensor("ident", [128, 128], BF16, kind="ExternalInput").ap()
    outT = nc.dram_tensor("outT", [c, lq], BF16, kind="ExternalOutput").ap()
    taps = {}
    if debug_taps:
        for nm, shp, dt in [
            ("dbg_qt", [128, 2, lq], BF16), ("dbg_kt", [128, 2, lkv], BF16),
            ("dbg_vaug", [128, lkv // 128, 4, 66], BF16),
            ("dbg_otn", [128, 2, lq], BF16),
            ("dbg_pt", [128, 2, 512], BF16),
            ("dbg_ot", [2, 128, 4, 65], F32),
            ("dbg_stg", [4, 128, 128], BF16),
        ]:
            taps[nm] = nc.dram_tensor(nm, shp, dt, kind="ExternalOutput").ap()

    ncc = c // 128          # contraction chunks for Q proj (8)
    nctx = ctx // 128       # contraction chunks for K/V proj (6)
    nit = lq // 512         # i tiles (4)
    njt = lkv // 128        # j chunks (16)
    npair = 2               # head pairs per core
    nct = c // 128          # out column tiles (8)
    nqc = 4                 # q sub-chunks of 128 per i-tile

    with tile.TileContext(nc) as tc:
        with (
            tc.tile_pool(name="big", bufs=1) as big,
            tc.tile_pool(name="wts", bufs=1) as wts,
            tc.tile_pool(name="acts", bufs=1) as acts,
            tc.tile_pool(name="pt", bufs=28) as ptp,
            tc.tile_pool(name="nrm", bufs=8) as nrm,
            tc.tile_pool(name="stg", bufs=8) as stgp,
            tc.tile_pool(name="osb", bufs=3) as osb,
            tc.tile_pool(name="st", bufs=4, space="PSUM") as stp,
            tc.tile_pool(name="ot", bufs=2, space="PSUM") as otp,
            tc.tile_pool(name="pp", bufs=2, space="PSUM") as ppp,
        ):
            # ---- persistent activations/weights in SBUF (bf16)
            qt = acts.tile([128, npair, lq], BF16, tag="qt")       # Q^T pair-stacked
            kt = acts.tile([128, npair, lkv], BF16, tag="kt")      # K^T pair-stacked
            vaug = acts.tile([128, njt, 4, 66], BF16, tag="vaug")  # [V_h | ones] per j-chunk
            otn = acts.tile([128, npair, lq], BF16, tag="otn")     # normalized O^T
            id_sb = acts.tile([128, 128], BF16, tag="id")

            x_sb = big.tile([128, ncc, lq], BF16, tag="x")
            y_sb = big.tile([128, nctx, lkv], BF16, tag="y")
            wq_sb = wts.tile([128, ncc, hd], BF16, tag="wq")
            wk_sb = wts.tile([128, nctx, hd], BF16, tag="wk")
            wv_sb = wts.tile([128, nctx, hd], BF16, tag="wv")
            wp_sb = wts.tile([128, npair, c], BF16, tag="wp")

            # ones column of vaug (static, no data deps)
            nc.vector.memset(vaug[:, :, :, 64:65], 1.0)

            # ---- input DMA (SP queue is FIFO): K/V-critical tensors first,
            # y in 4 column chunks so K proj can start after the first one.
            yv = yT.rearrange("(cc p) l -> p cc l", p=128)
            xv = xT.rearrange("(cc p) l -> p cc l", p=128)
            nc.sync.dma_start(out=wk_sb, in_=wk.rearrange("(cc p) h -> p cc h", p=128))
            nc.sync.dma_start(out=y_sb[:, :, 0:512], in_=yv[:, :, 0:512])
            nc.sync.dma_start(out=wq_sb, in_=wq.rearrange("(cc p) h -> p cc h", p=128))
            nc.sync.dma_start(out=x_sb[:, :, 0:512], in_=xv[:, :, 0:512])
            nc.sync.dma_start(out=y_sb[:, :, 512:1024], in_=yv[:, :, 512:1024])
            nc.sync.dma_start(out=wv_sb, in_=wv.rearrange("(cc p) h -> p cc h", p=128))
            for t in range(2, nit):
                nc.sync.dma_start(out=y_sb[:, :, t * 512:(t + 1) * 512],
                                  in_=yv[:, :, t * 512:(t + 1) * 512])
            nc.sync.dma_start(out=x_sb[:, :, 512:lq], in_=xv[:, :, 512:lq])
            nc.sync.dma_start(out=id_sb, in_=ident)
            nc.sync.dma_start(out=wp_sb, in_=wp.rearrange("(r p) o -> p r o", p=128))

            # ---- K/V projection pieces (upfront: K-tile0 + V jt 0..3;
            # the rest drains through the phase C thunk queue just ahead
            # of consumption)
            def k_tile(t):
                for pair in range(npair):
                    ps = ppp.tile([128, 512], F32, tag="ps")
                    for cc in range(nctx):
                        nc.tensor.matmul(
                            ps[:],
                            wk_sb[:, cc, pair * 128:(pair + 1) * 128],
                            y_sb[:, cc, t * 512:(t + 1) * 512],
                            start=(cc == 0), stop=(cc == nctx - 1))
                    nc.vector.tensor_copy(kt[:, pair, t * 512:(t + 1) * 512], ps[:])

            def v_tile(jt):
                ps = ppp.tile([128, 512], F32, tag="ps")
                for cc in range(nctx):
                    nc.tensor.matmul(
                        ps[:, 0:256],
                        y_sb[:, cc, jt * 128:(jt + 1) * 128],
                        wv_sb[:, cc, :],
                        start=(cc == 0), stop=(cc == nctx - 1))
                nc.vector.tensor_copy(
                    vaug[:, jt, :, 0:64],
                    ps[:, 0:256].rearrange("p (h d) -> p h d", d=64))

            k_tile(0)

            # ---- Q projection (one pair-tile per thunk; it=0 up front)
            def q_proj_half(it, pair):
                ps = ppp.tile([128, 512], F32, tag="ps")
                for cc in range(ncc):
                    nc.tensor.matmul(
                        ps[:],
                        wq_sb[:, cc, pair * 128:(pair + 1) * 128],
                        x_sb[:, cc, it * 512:(it + 1) * 512],
                        start=(cc == 0), stop=(cc == ncc - 1))
                nc.vector.tensor_copy(qt[:, pair, it * 512:(it + 1) * 512], ps[:])

            # ---- output projection, one ct tile per thunk. All 8 ct tiles
            # of an i-tile stage into one SBUF buffer and leave in a single
            # DMA (8 separate stores would serialize on HWDGE descriptor
            # processing). The PSUM->SBUF copy alternates DVE/Act so the
            # drain of the last i-tile is not DVE-serial.
            osb_tiles = {}

            def d_tile(it, ct):
                last = it == nit - 1
                if last and ct % 2 == 1:
                    # the drain has no live score tiles; alternating pools
                    # doubles the effective PSUM ring for the final i-tile
                    ps = stp.tile([128, 512], F32, tag="st", name="dps")
                    ps = ps[:]
                else:
                    ps = ppp.tile([128, 512], F32, tag="ps")
                for pair in range(npair):
                    nc.tensor.matmul(
                        ps,
                        wp_sb[:, pair, ct * 128:(ct + 1) * 128],
                        otn[:, pair, it * 512:(it + 1) * 512],
                        start=(pair == 0), stop=(pair == npair - 1))
                if ct == 0:
                    osb_tiles[it] = osb.tile([128, nct, 512], BF16, tag="osb",
                                             name=f"osb{it}")
                o_sb = osb_tiles[it]
                if last and ct % 2 == 0:
                    nc.scalar.copy(o_sb[:, ct, :], ps)
                else:
                    nc.vector.tensor_copy(o_sb[:, ct, :], ps)
                sl = slice(it * 512, (it + 1) * 512)
                if last and ct % 2 == 1:
                    # stream the drain out in quarters so the final DMA is
                    # short; earlier i-tiles leave in one bulk store
                    nc.sync.dma_start(
                        out=outT[(ct - 1) * 128:(ct + 1) * 128, sl].rearrange(
                            "(ct p) l -> p ct l", p=128),
                        in_=o_sb[:, ct - 1:ct + 1, :])
                elif not last and ct == nct - 1:
                    nc.sync.dma_start(
                        out=outT[:, sl].rearrange("(ct p) l -> p ct l", p=128),
                        in_=o_sb[:])

            q_proj_half(0, 0)
            k_tile(1)
            v_tile(0)
            v_tile(1)
            q_proj_half(0, 1)

            # ---- phase C: attention per (pair, i-tile), pipelined over jt
            # and across phases. `prev` carries the previous phase's last PV
            # steps + normalization; `extra` is a queue of tail thunks
            # (transposes, Q proj halves, D tiles) drained one per jt step.
            def c_phase(pair, it, prev, extra):
                ot_a = otp.tile([128, nqc, 65], F32, tag="ot")
                ot_b = otp.tile([128, nqc, 65], F32, tag="ot")
                ots = (ot_a, ot_b)
                pts = {}
                sts = {}

                def scores(jt, h):
                    # one head per score tile: a 1-bank PSUM tile, so the
                    # same 4 banks give a 4-slot ring and each st fence hop
                    # is a 612ns single-head exp instead of 1038ns
                    st = stp.tile([128, 512], F32, tag="st")
                    nc.tensor.matmul(
                        st[:],
                        kt[64 * h:64 * h + 64, pair, jt * 128:(jt + 1) * 128],
                        qt[64 * h:64 * h + 64, pair, it * 512:(it + 1) * 512],
                        start=True, stop=True)
                    sts[jt, h] = st

                def expj(jt, h):
                    pt = ptp.tile([128, 512], BF16, tag="pt")
                    if jt in FEXP_JT:
                        nc.vector.tensor_scalar(
                            pt[:].bitcast(I16), sts[jt, h][:], FEXP_C1,
                            FEXP_B0, mybir.AluOpType.mult, mybir.AluOpType.add)
                    else:
                        nc.scalar.activation(
                            pt[:], sts[jt, h][:],
                            mybir.ActivationFunctionType.Exp, scale=SCALE)
                    pts[jt, h] = pt

                def pv(jt):
                    # start=True zeroes the WHOLE PSUM bank on hardware, so
                    # only the first matmul into each ot bank may use it; the
                    # other q-chunks accumulate onto the bank-wide zero.
                    for h in range(2):
                        for qc in range(nqc):
                            nc.tensor.matmul(
                                ots[h][:, qc, 0:65],
                                pts[jt, h][:, qc * 128:(qc + 1) * 128],
                                vaug[:, jt, 2 * pair + h, 0:65],
                                start=(jt == 0 and qc == 0),
                                stop=(jt == njt - 1))

                def norm():
                    """Reciprocal + per-partition scale -> bf16 stg tiles;
                    returns PE transpose/copy thunks (run in the NEXT phase).
                    Head a (slot ot_a) first so its PSUM slot frees earliest."""
                    if debug_taps and pair == 0 and it == 0:
                        for h in range(2):
                            dbg = stgp.tile([128, 4, 65], F32, tag="dbgot",
                                            name=f"dbg{h}")
                            nc.vector.tensor_copy(dbg[:], ots[h][:])
                            nc.sync.dma_start(out=taps["dbg_ot"][h], in_=dbg[:])
                    stgs = [stgp.tile([128, 128], BF16, tag="stg",
                                      name=f"stg{qc}")
                            for qc in range(nqc)]
                    for h in range(2):
                        rcp = nrm.tile([128, nqc], F32, tag="rcp")
                        nc.vector.reciprocal(
                            out=rcp[:].rearrange("p (q o) -> p q o", o=1),
                            in_=ots[h][:, :, 64:65])
                        for qc in range(nqc):
                            nc.vector.tensor_scalar(
                                stgs[qc][:, h * 64:(h + 1) * 64],
                                ots[h][:, qc, 0:64],
                                rcp[:, qc:qc + 1], None, mybir.AluOpType.mult)
                    if debug_taps and pair == 0 and it == 0:
                        for qc in range(nqc):
                            nc.sync.dma_start(out=taps["dbg_stg"][qc],
                                              in_=stgs[qc][:])

                    def mk(qc):
                        def tr_thunk():
                            ps = ppp.tile([128, 512], F32, tag="ps")
                            tr = ps[:, 0:64].bitcast(BF16)
                            nc.tensor.transpose(tr, stgs[qc][:], id_sb[:])
                            nc.vector.tensor_copy(
                                otn[:, pair, it * 512 + qc * 128:
                                    it * 512 + (qc + 1) * 128],
                                tr)
                        return tr_thunk
                    return [mk(qc) for qc in range(nqc)]

                # prologue of the jt pipeline
                for jt in range(PVLAG):
                    scores(jt, 0)
                    expj(jt, 0)
                    scores(jt, 1)
                    expj(jt, 1)
                if prev is not None:
                    prev["pv_tail"]()          # prev phase pv(13..15)
                    extra[0:0] = prev["norm"]()  # prev normalization + its trs
                for jt in range(PVLAG, njt):
                    scores(jt, 0)
                    expj(jt, 0)
                    scores(jt, 1)
                    expj(jt, 1)
                    pv(jt - PVLAG)
                    if extra:
                        extra.pop(0)()

                def pv_tail():
                    for jt in range(njt - PVLAG, njt):
                        pv(jt)
                return {"pv_tail": pv_tail, "norm": norm}

            prev = None
            # remaining K/V projection tiles drain through the thunk queue
            # of phase (0,0) -- drains happen at jt slots PVLAG..15, one per
            # slot, AFTER that slot's pv. Deadlines (hand-checked for
            # PVLAG=7): K_t must drain before scores(4t) is emitted, V_j
            # before pv(j) at slot j+PVLAG; K0/K1/V0/V1 have deadlines
            # earlier than the first drain slot and run upfront above.
            def vv(*jts):
                return lambda: [v_tile(j) for j in jts]
            extra = [
                lambda: k_tile(2), vv(2, 3), vv(4, 5),
                lambda: k_tile(3), vv(6, 7), vv(8, 9),
                vv(10, 11), vv(12, 13), vv(14, 15),
            ]
            for it in range(nit):
                prev = c_phase(0, it, prev, extra)
                if it < nit - 1:
                    # front-insert: qt(it+1) must be ready before phase (0,it+1)
                    extra[0:0] = [lambda it=it, p=p: q_proj_half(it + 1, p)
                                  for p in range(npair)]
                if it >= 1:
                    # append here (not after phase (1,it)) so the 8 tiles
                    # drain in-band; otn(it-1) completed via the transposes
                    # drained during phase (0,it) above
                    extra += [lambda it=it, ct=ct: d_tile(it - 1, ct)
                              for ct in range(nct)]
                prev = c_phase(1, it, prev, extra)
            # drain: last phase tail + remaining thunks + final out tiles
            prev["pv_tail"]()
            for thunk in extra + prev["norm"]():
                thunk()
            for ct in range(nct):
                d_tile(nit - 1, ct)

            if debug_taps:
                nc.sync.dma_start(out=taps["dbg_qt"], in_=qt[:])
                nc.sync.dma_start(out=taps["dbg_kt"], in_=kt[:])
                nc.sync.dma_start(out=taps["dbg_vaug"], in_=vaug[:])
                nc.sync.dma_start(out=taps["dbg_otn"], in_=otn[:])

    nc.compile()
    return nc


_NC_CACHE = {}


def _get_nc():
    if "nc" not in _NC_CACHE:
        _NC_CACHE["nc"] = build_kernel()
    return _NC_CACHE["nc"]


def make_in_maps(x, y, Wq, Wkv, Wproj):
    """Host-side sharding: core = b * 4 + hg (hg = 4-head group)."""
    x = np.asarray(x, dtype=np.float32)
    y = np.asarray(y, dtype=np.float32)
    Wq = np.asarray(Wq, dtype=np.float32)
    Wkv = np.asarray(Wkv, dtype=np.float32).reshape(CTX, 2, H, DK)
    Wproj = np.asarray(Wproj, dtype=np.float32)
    ident = np.eye(128, dtype=np.float32).astype(BF16NP)

    in_maps = []
    for core in range(8):
        b, hg = core // 4, core % 4
        hs = slice(4 * hg, 4 * hg + 4)
        in_maps.append({
            "xT": np.ascontiguousarray(x[b].T).astype(BF16NP),
            "yT": np.ascontiguousarray(y[b].T).astype(BF16NP),
            "wq": np.ascontiguousarray(
                Wq[:, 4 * hg * DK:(4 * hg + 4) * DK]).astype(BF16NP),
            "wk": np.ascontiguousarray(
                Wkv[:, 0, hs, :].reshape(CTX, 4 * DK)).astype(BF16NP),
            "wv": np.ascontiguousarray(
                Wkv[:, 1, hs, :].reshape(CTX, 4 * DK)).astype(BF16NP),
            "wp": np.ascontiguousarray(
                Wproj[4 * hg * DK:(4 * hg + 4) * DK, :]).astype(BF16NP),
            "ident": ident,
        })
    return in_maps


def kernel(x, y, Wq, Wkv, Wproj, bproj):
    nc = _get_nc()
    in_maps = make_in_maps(x, y, Wq, Wkv, Wproj)
    res = run_bass_kernel_spmd(nc, in_maps, core_ids=list(range(8)))
    bproj = np.asarray(bproj, dtype=np.float32)
    out = np.empty((B, LQ, C), dtype=np.float32)
    for b in range(B):
        acc = res.results[4 * b]["outT"].astype(np.float32).copy()
        for hg in range(1, 4):
            acc += res.results[4 * b + hg]["outT"]
        out[b] = acc.T + bproj
    return out
